# revision 10
# baseline (speedup 1.0000x reference)
"""AudioCondTransformerEncoderLayer on 8 Trainium2 NeuronCores.

Strategy (hardcoded for TM=TA=512, B=32, D=1024, H=16, DFF=4096):
  - Data-parallel over batch: 4 batch elements per core, 8 cores.
  - Activations live feature-major ([d, t]) on-chip so every projection is a
    natural PE matmul with the contraction dim on partitions.
  - Residual-stream matmuls run in float32r (full PE rate at N>=256,
    ~1.5e-4 rel err); attention internals (q/k/v/expS/o + out-proj weights)
    are bf16 to fit SBUF (softmax renormalizes, so the error stays ~1e-3).
  - Attention per (b, h): S^T = k_h^T.T @ q_h^T chunks ([tk, tq]); exp on ACT;
    softmax denominator via ones-column matmul reduction; reciprocal broadcast
    across partitions via a rank-1 PE outer product; attn@V with lhsT = v
    (token-major) giving o^T feature-major, normalized on DVE during PSUM
    copy-out.
  - Cross-attention bias (temporal + beat, batch-independent) is computed on
    host, uploaded once, and accumulated into the score PSUM with an
    identity-weight matmul.
  - LayerNorm in feature-major: partition sums via ones-matmuls, sqrt on ACT +
    reciprocal on DVE, mean/invstd broadcast via PE outer products, apply via
    DVE with per-partition gain/bias tensor_scalar.
  - FFN: lin1 (f32r) -> exact Gelu on ACT -> hT bf16 -> lin2 (bf16).
"""

import numpy as np

# ---------------------------------------------------------------------------
# Problem constants
# ---------------------------------------------------------------------------
D = 1024
H = 16
HD = 64
TM = 512
TA = 512
B = 32
DFF = 4096
NCORES = 8
BPC = B // NCORES          # batch elems per core
SIGMA = 4.0
BW = 2.0
LN_EPS = 1e-5
KD = D // 128              # 8 d-chunks
KF = DFF // 128            # 32 ff-chunks
TCH = TM // 128            # 4 token chunks

_CACHE = {}


# ---------------------------------------------------------------------------
# Walrus workaround: this container's walrus build rejects >1 sync-wait per
# instruction. Split excess waits onto preceding same-engine NOPs, and move
# the tail drain's waits onto SP NOPs.
# ---------------------------------------------------------------------------
def _install_patches():
    if _CACHE.get("patched"):
        return
    import concourse.mybir as mybir
    import concourse.tile as tile
    import concourse.tile_utils as tile_utils
    from concourse.vector_clock import ScopedClock

    tile_utils.max_sbuf_usage = 208 * 1024

    _orig_commit = tile.TileContext._commit_instruction

    def _split_commit(self, inst, lazy_reg_writes=True):
        si = inst.sync_info
        if (
            si is not None
            and len(si.on_wait) > 1
            and inst.engine != mybir.EngineType.Unassigned
        ):
            waits = list(si.on_wait)
            inst.sync_info = mybir.SyncInfo(
                on_wait=waits[:1], on_update=list(si.on_update)
            )
            for w in waits[1:]:
                nop = mybir.InstNoOp(
                    name=self.nc.get_next_instruction_name(),
                    ins=[],
                    outs=[],
                    engine=inst.engine,
                    sync_info=mybir.SyncInfo(on_wait=[w], on_update=[]),
                )
                nop.debug = inst.debug
                _orig_commit(self, nop, lazy_reg_writes=False)
        return _orig_commit(self, inst, lazy_reg_writes=lazy_reg_writes)

    tile.TileContext._commit_instruction = _split_commit

    def _patched_drain_and_barrier(self, tick_clock, wait_clock):
        carrier = self.nc.sync.nop(nofuse=True)
        wait_clock.add_sem_waits(
            carrier.ins, ScopedClock({None: tick_clock.global_clock})
        )
        si = carrier.ins.sync_info
        if si is not None and len(si.on_wait) > 1:
            waits = list(si.on_wait)
            carrier.ins.sync_info = mybir.SyncInfo(
                on_wait=waits[:1], on_update=list(si.on_update)
            )
            for w in waits[1:]:
                extra = self.nc.sync.nop(nofuse=True)
                extra.ins.sync_info = mybir.SyncInfo(on_wait=[w], on_update=[])
        self.nc.sync.drain()
        self.nc.all_engine_barrier()
        popped = self.nc._tile_sem_poison_stack.pop()
        assert popped is self._sem_poison
        self.nc.clear_and_free_semaphores(list(self.sems.allocated().values()))
        self.nc.all_engine_barrier()

    tile.TileContext._drain_and_barrier = _patched_drain_and_barrier
    _CACHE["patched"] = True


# ---------------------------------------------------------------------------
# Device module
# ---------------------------------------------------------------------------
def _build_module():
    from contextlib import ExitStack

    import concourse.bass as bass
    import concourse.mybir as mybir
    import concourse.tile as tile

    f32 = mybir.dt.float32
    f32r = mybir.dt.float32r
    bf16 = mybir.dt.bfloat16
    AF = mybir.ActivationFunctionType
    OP = mybir.AluOpType

    nc = bass.Bass()

    def din(name, shape, dt=f32):
        return nc.dram_tensor(name, shape, dt, kind="ExternalInput")

    xin = din("xin", (BPC, KD, 128, TM))
    ain = din("ain", (BPC, KD, 128, TA))
    wqk_sa = din("wqk_sa", (16, 128, KD, 128))
    wv_sa = din("wv_sa", (4, 128, KD, 256))
    wo_sa = din("wo_sa", (8, 128, KD, 128), bf16)
    wqk_ca = din("wqk_ca", (16, 128, KD, 128))
    wv_ca = din("wv_ca", (4, 128, KD, 256))
    wo_ca = din("wo_ca", (8, 128, KD, 128), bf16)
    w1 = din("w1", (KF, 128, KD, 128))
    w2 = din("w2", (8, 2, 128, KF // 2, 128), bf16)
    bqk_sa = din("bqk_sa", (128, 16))
    bo_sa = din("bo_sa", (128, 8))
    bqk_ca = din("bqk_ca", (128, 16))
    bo_ca = din("bo_ca", (128, 8))
    b1 = din("b1", (128, KF))
    b2 = din("b2", (128, 8))
    n1g = din("n1g", (128, 8))
    n1b = din("n1b", (128, 8))
    ncg = din("ncg", (128, 8))
    ncb = din("ncb", (128, 8))
    n2g = din("n2g", (128, 8))
    n2b = din("n2b", (128, 8))
    biasT = din("biasT", (TCH, 128, TM))
    ident = din("ident", (128, 128))
    onescol = din("onescol", (128, 1))
    onesrow = din("onesrow", (1, 128))

    out = nc.dram_tensor("out", (BPC, KD, 128, TM), f32, kind="ExternalOutput")

    with tile.TileContext(nc) as tc, ExitStack() as ctx:
        cpool = ctx.enter_context(tc.tile_pool(name="consts", bufs=1))
        # f32r activation tiles, 16KB/partition each, time-shared via one tag
        actp = ctx.enter_context(tc.tile_pool(name="acts", bufs=3))
        # bf16 attention tensors
        qkp = ctx.enter_context(tc.tile_pool(name="qkp", bufs=1))
        vp = ctx.enter_context(tc.tile_pool(name="vp", bufs=1))
        onp = ctx.enter_context(tc.tile_pool(name="onp", bufs=1))
        hp = ctx.enter_context(tc.tile_pool(name="hpool", bufs=1))
        expp = ctx.enter_context(tc.tile_pool(name="expS", bufs=6))
        # weight streams
        wp = ctx.enter_context(tc.tile_pool(name="wstream", bufs=3))
        wvp = ctx.enter_context(tc.tile_pool(name="wvstream", bufs=2))
        wbp = ctx.enter_context(tc.tile_pool(name="wbfstream", bufs=3))
        w2p = ctx.enter_context(tc.tile_pool(name="w2stream", bufs=2))
        # misc
        smp = ctx.enter_context(tc.tile_pool(name="small", bufs=4))
        smrp = ctx.enter_context(tc.tile_pool(name="smallr", bufs=2))
        bcp = ctx.enter_context(tc.tile_pool(name="bcast", bufs=2))
        tmpp = ctx.enter_context(tc.tile_pool(name="tmp", bufs=2))
        sqp = ctx.enter_context(tc.tile_pool(name="sq", bufs=2))
        scrp = ctx.enter_context(tc.tile_pool(name="scratch", bufs=2))
        # psum
        psa = ctx.enter_context(tc.tile_pool(name="psa", bufs=2, space="PSUM"))
        pss = ctx.enter_context(tc.tile_pool(name="pss", bufs=2, space="PSUM"))
        scorep = ctx.enter_context(tc.tile_pool(name="scorep", bufs=2, space="PSUM"))

        # --- constants -----------------------------------------------------
        ident_r = cpool.tile([128, 128], f32r, name="ident_r")
        nc.sync.dma_start(ident_r[:], ident[:, :].bitcast(f32r))
        ones_c = cpool.tile([128, 1], f32r, name="ones_c")
        nc.sync.dma_start(ones_c[:], onescol[:, :].bitcast(f32r))
        ones_cb = cpool.tile([128, 1], bf16, name="ones_cb")
        nc.vector.tensor_copy(ones_cb[:], ones_c[:].bitcast(f32))
        ones_r = cpool.tile([1, 128], f32r, name="ones_r")
        nc.sync.dma_start(ones_r[:], onesrow[:, :].bitcast(f32r))
        biasT_r = cpool.tile([128, TCH, TM], f32r, name="biasT_r")
        nc.sync.dma_start(
            biasT_r[:], biasT[:, :, :].rearrange("c p t -> p c t").bitcast(f32r))
        eps_t = cpool.tile([1, 1], f32, name="eps_t")
        nc.vector.memset(eps_t[:], LN_EPS)

        def load_pp(dram, n):
            t = cpool.tile([128, n], f32, name=dram.name + "_t")
            nc.sync.dma_start(t[:], dram[:, :])
            return t

        bqk_sa_t = load_pp(bqk_sa, 16)
        bo_sa_t = load_pp(bo_sa, 8)
        bqk_ca_t = load_pp(bqk_ca, 16)
        bo_ca_t = load_pp(bo_ca, 8)
        b1_t = load_pp(b1, KF)
        b2_t = load_pp(b2, 8)
        n1g_t, n1b_t = load_pp(n1g, 8), load_pp(n1b, 8)
        ncg_t, ncb_t = load_pp(ncg, 8), load_pp(ncb, 8)
        n2g_t, n2b_t = load_pp(n2g, 8), load_pp(n2b, 8)

        # --- helpers -------------------------------------------------------
        def ln(y, g_t, b_t, dst):
            """LayerNorm over the partition (feature) axis of y [128, KD, T]."""
            ps_s = pss.tile([1, TM], f32, tag="sps", name="ps_s")
            for k in range(KD):
                nc.tensor.matmul(ps_s[:], ones_c[:], y[:, k],
                                 start=(k == 0), stop=(k == KD - 1))
            mean = smp.tile([1, TM], f32, tag="sm", name="mean")
            nc.scalar.mul(mean[:], ps_s[:], 1.0 / D)
            ps_q = pss.tile([1, TM], f32, tag="sps", name="ps_q")
            for k in range(KD):
                sq = sqp.tile([128, TM], f32r, tag="sq", name="sq")
                nc.scalar.square(sq[:], y[:, k].bitcast(f32))
                nc.tensor.matmul(ps_q[:], ones_c[:], sq[:],
                                 start=(k == 0), stop=(k == KD - 1))
            msq = smp.tile([1, TM], f32, tag="sm", name="msq")
            nc.scalar.mul(msq[:], ps_q[:], 1.0 / D)
            var = smp.tile([1, TM], f32, tag="sm", name="var")
            nc.vector.tensor_tensor(var[:], mean[:], mean[:], OP.mult)
            nc.vector.tensor_tensor(var[:], msq[:], var[:], OP.subtract)
            sd = smp.tile([1, TM], f32, tag="sm", name="sd")
            nc.scalar.activation(sd[:], var[:], AF.Sqrt, bias=eps_t[:])
            invr = smrp.tile([1, TM], f32r, tag="smr", name="invr")
            c2r = smrp.tile([1, TM], f32r, tag="smr", name="c2r")
            with nc.allow_low_precision(reason="f32r rounding is fine here"):
                nc.vector.reciprocal(invr[:], sd[:])
                nc.vector.tensor_tensor(c2r[:], mean[:], invr[:].bitcast(f32),
                                        OP.mult)
            bA = psa.tile([128, TM], f32, tag="mm", name="bA")
            nc.tensor.matmul(bA[:], ones_r[:], invr[:], start=True, stop=True)
            bC = psa.tile([128, TM], f32, tag="mm", name="bC")
            nc.tensor.matmul(bC[:], ones_r[:], c2r[:], start=True, stop=True)
            bAs = bcp.tile([128, TM], f32, tag="bcs", name="bAs")
            nc.scalar.copy(bAs[:], bA[:])
            bCs = bcp.tile([128, TM], f32, tag="bcs", name="bCs")
            nc.scalar.copy(bCs[:], bC[:])
            for k in range(KD):
                t1 = tmpp.tile([128, TM], f32, tag="t1", name="t1")
                nc.vector.tensor_tensor(t1[:], y[:, k].bitcast(f32), bAs[:], OP.mult)
                nc.vector.tensor_tensor(t1[:], t1[:], bCs[:], OP.subtract)
                nc.scalar.activation(dst[:, k], t1[:], AF.Identity,
                                      bias=b_t[:, k:k + 1], scale=g_t[:, k:k + 1])

        def attention(qkT, v, onT, with_bias):
            """One batch elem's attention. qkT [128,16,T] bf16 (q 0..7, k 8..15),
            v [128,TCH,16*65] bf16 token-major with a ones column per head,
            onT [128,KD,T] bf16 out (normalized o^T feature-major).
            Head pairs (2hp, 2hp+1) live on partition halves 0-63 / 64-127, so
            their K=64 score matmuls run concurrently on disjoint PE row
            strips."""
            for hp in range(H // 2):
                q0 = qkT[0:64, hp, :]
                q1 = qkT[64:128, hp, :]
                exps = {0: [], 1: []}
                for cp in range(2):
                    sps0 = scorep.tile([128, 2, TM], f32, tag="sc", name="sps0")
                    sps1 = scorep.tile([128, 2, TM], f32, tag="sc", name="sps1")
                    for j in range(2):
                        c = 2 * cp + j
                        k0 = qkT[0:64, 8 + hp, 128 * c:128 * c + 128]
                        k1 = qkT[64:128, 8 + hp, 128 * c:128 * c + 128]
                        if with_bias:
                            nc.tensor.matmul(sps0[:, j], k0, q0, start=True, stop=False)
                            nc.tensor.matmul(sps1[:, j], k1, q1, start=True, stop=False)
                            nc.tensor.matmul(sps0[:, j], ident_r[:], biasT_r[:, c, :],
                                             start=False, stop=True)
                            nc.tensor.matmul(sps1[:, j], ident_r[:], biasT_r[:, c, :],
                                             start=False, stop=True)
                        else:
                            nc.tensor.matmul(sps0[:, j], k0, q0, start=True, stop=True)
                            nc.tensor.matmul(sps1[:, j], k1, q1, start=True, stop=True)
                    e0 = expp.tile([128, 2, TM], bf16, tag="e", name="e0")
                    nc.scalar.activation(e0[:], sps0[:], AF.Exp)
                    exps[0].append(e0)
                    e1 = expp.tile([128, 2, TM], bf16, tag="e", name="e1")
                    nc.scalar.activation(e1[:], sps1[:], AF.Exp)
                    exps[1].append(e1)
                for par in (0, 1):
                    h = 2 * hp + par
                    ex = exps[par]
                    op = pss.tile([65, TM], f32, tag="sps", name="op")
                    for c in range(TCH):
                        nc.tensor.matmul(op[:], v[:, c, 65 * h:65 * h + 65],
                                         ex[c // 2][:, c % 2, :],
                                         start=(c == 0), stop=(c == TCH - 1))
                    rr = smrp.tile([1, TM], f32r, tag="smr", name="rr")
                    nc.vector.tensor_copy(rr[:], op[64:65, :])
                    bc = psa.tile([64, TM], f32, tag="mm", name="bc")
                    nc.tensor.matmul(bc[:], ones_r[0:1, 0:64], rr[:],
                                     start=True, stop=True)
                    bcs = bcp.tile([64, TM], f32, tag="bcs", name="bcs")
                    nc.vector.reciprocal(bcs[:], bc[:])
                    if par == 0:
                        nc.vector.tensor_tensor(onT[0:64, hp, :], op[0:64, :],
                                                bcs[:], OP.mult)
                    else:
                        sc = scrp.tile([64, TM], bf16, tag="shift", name="sc")
                        nc.vector.tensor_tensor(sc[:], op[0:64, :], bcs[:], OP.mult)
                        nc.sync.dma_start(onT[64:128, hp, :], sc[:])

        def v_proj(wdram, srcT, vdst):
            # ones columns for the softmax-denominator trick
            nc.vector.memset(vdst[:, :, 64::65], 1.0)
            for qt in range(4):
                wvt = wvp.tile([128, KD, 256], f32r, tag="wv", name="wvt")
                nc.sync.dma_start(
                    wvt[:], wdram[qt].bitcast(f32r))
                for tch in range(TCH):
                    ps = psa.tile([128, 256], f32, tag="mm", name="vps")
                    for k in range(KD):
                        nc.tensor.matmul(
                            ps[:], srcT[:, k, 128 * tch:128 * tch + 128],
                            wvt[:, k], start=(k == 0), stop=(k == KD - 1))
                    dst = vdst[:, tch, 65 * 4 * qt:65 * 4 * qt + 260]
                    dst = dst.rearrange("p (h f) -> p h f", f=65)[:, :, 0:64]
                    nc.vector.tensor_copy(dst, ps[:].rearrange("p (h f) -> p h f", f=64))

        def out_proj_res_ln(wdram, bias_t, onT, resT, g_t, bt_t, dstT):
            xres = actp.tile([128, KD, TM], f32r, tag="act8", name="xres")
            for ec in range(8):
                wt = wbp.tile([128, KD, 128], bf16, tag="wbf", name="wot")
                nc.sync.dma_start(wt[:], wdram[ec])
                ps = psa.tile([128, TM], f32, tag="mm", name="ops")
                for k in range(KD):
                    nc.tensor.matmul(ps[:], wt[:, k], onT[:, k],
                                     start=(k == 0), stop=(k == KD - 1))
                t1 = tmpp.tile([128, TM], f32, tag="t1", name="t1o")
                nc.vector.tensor_scalar_add(t1[:], ps[:], bias_t[:, ec:ec + 1])
                nc.vector.tensor_tensor(xres[:, ec], t1[:],
                                        resT[:, ec].bitcast(f32), OP.add)
            ln(xres, g_t, bt_t, dstT)

        # --- main loop over the core's 4 batch elems ----------------------
        for b in range(BPC):
            xT = actp.tile([128, KD, TM], f32r, tag="act8", name="xT")
            for k in range(KD):
                nc.sync.dma_start(xT[:, k], xin[b, k].bitcast(f32r))

            # A: SA projections
            qkT = qkp.tile([128, 16, TM], bf16, tag="qkT", name="qkT")
            for ec in range(16):
                wt = wp.tile([128, KD, 128], f32r, tag="w8x128", name="wqkt")
                nc.sync.dma_start(wt[:], wqk_sa[ec].bitcast(f32r))
                ps = psa.tile([128, TM], f32, tag="mm", name="qkps")
                for k in range(KD):
                    nc.tensor.matmul(ps[:], wt[:, k], xT[:, k],
                                     start=(k == 0), stop=(k == KD - 1))
                nc.vector.tensor_scalar_add(qkT[:, ec], ps[:],
                                            bqk_sa_t[:, ec:ec + 1])
            v = vp.tile([128, TCH, H * 65], bf16, tag="vT", name="vT")
            v_proj(wv_sa, xT, v)

            # B: SA attention
            onT = onp.tile([128, KD, TM], bf16, tag="onT", name="onT")
            attention(qkT, v, onT, with_bias=False)

            # C: SA out-proj + residual + LN1
            x1T = actp.tile([128, KD, TM], f32r, tag="act8", name="x1T")
            out_proj_res_ln(wo_sa, bo_sa_t, onT, xT, n1g_t, n1b_t, x1T)

            # D: CA projections (q from x1, k/v from audio)
            aT = actp.tile([128, KD, TA], f32r, tag="act8", name="aT")
            for k in range(KD):
                nc.sync.dma_start(aT[:, k], ain[b, k].bitcast(f32r))
            qkT2 = qkp.tile([128, 16, TM], bf16, tag="qkT", name="qkT2")
            for ec in list(range(8, 16)) + list(range(8)):
                wt = wp.tile([128, KD, 128], f32r, tag="w8x128", name="wqkt2")
                nc.sync.dma_start(wt[:], wqk_ca[ec].bitcast(f32r))
                src = x1T if ec < 8 else aT
                ps = psa.tile([128, TM], f32, tag="mm", name="qkps2")
                for k in range(KD):
                    nc.tensor.matmul(ps[:], wt[:, k], src[:, k],
                                     start=(k == 0), stop=(k == KD - 1))
                nc.vector.tensor_scalar_add(qkT2[:, ec], ps[:],
                                            bqk_ca_t[:, ec:ec + 1])
            v2 = vp.tile([128, TCH, H * 65], bf16, tag="vT", name="v2T")
            v_proj(wv_ca, aT, v2)

            # E: CA attention (with host-computed temporal+beat bias)
            onT2 = onp.tile([128, KD, TM], bf16, tag="onT", name="onT2")
            attention(qkT2, v2, onT2, with_bias=True)

            # F: CA out-proj (tanh(gate) folded on host) + residual + LNc
            x2T = actp.tile([128, KD, TM], f32r, tag="act8", name="x2T")
            out_proj_res_ln(wo_ca, bo_ca_t, onT2, x1T, ncg_t, ncb_t, x2T)

            # G: lin1 + gelu -> hT (bf16)
            hT = hp.tile([128, KF, TM], bf16, tag="hT", name="hT")
            for fc in range(KF):
                wt = wp.tile([128, KD, 128], f32r, tag="w8x128", name="w1t")
                nc.sync.dma_start(wt[:], w1[fc].bitcast(f32r))
                ps = psa.tile([128, TM], f32, tag="mm", name="hps")
                for k in range(KD):
                    nc.tensor.matmul(ps[:], wt[:, k], x2T[:, k],
                                     start=(k == 0), stop=(k == KD - 1))
                nc.scalar.activation(hT[:, fc], ps[:], AF.Gelu,
                                     bias=b1_t[:, fc:fc + 1])

            # H: lin2 + residual + LN2 -> out
            xres2 = actp.tile([128, KD, TM], f32r, tag="act8", name="xres2")
            for ec in range(8):
                ps = psa.tile([128, TM], f32, tag="mm", name="fps")
                for half in range(2):
                    w2t = w2p.tile([128, KF // 2, 128], bf16, tag="w2t", name="w2t")
                    nc.sync.dma_start(w2t[:], w2[ec, half])
                    for fo in range(KF // 2):
                        fg = half * (KF // 2) + fo
                        nc.tensor.matmul(ps[:], w2t[:, fo], hT[:, fg],
                                         start=(fg == 0), stop=(fg == KF - 1))
                t1 = tmpp.tile([128, TM], f32, tag="t1", name="t1f")
                nc.vector.tensor_scalar_add(t1[:], ps[:], b2_t[:, ec:ec + 1])
                nc.vector.tensor_tensor(xres2[:, ec], t1[:],
                                        x2T[:, ec].bitcast(f32), OP.add)
            outT = actp.tile([128, KD, TM], f32, tag="act8", name="outT")
            ln(xres2, n2g_t, n2b_t, outT)
            for k in range(KD):
                nc.sync.dma_start(out[b, k], outT[:, k])

    return nc


def _get_module():
    if "nc" not in _CACHE:
        _install_patches()
        _CACHE["nc"] = _build_module()
    return _CACHE["nc"]


# ---------------------------------------------------------------------------
# Host-side prep + execution
# ---------------------------------------------------------------------------
def _beat_bias(beats):
    beats = np.asarray(beats).astype(np.int64).ravel()
    bias = np.zeros(TA, np.float32)
    l_idx = np.where(beats > 0, beats - 1, 0)
    l_val = np.where(beats > 0, BW * 0.5, 0.0).astype(np.float32)
    r_idx = np.where(beats < TA - 1, beats + 1, TA - 1)
    r_val = np.where(beats < TA - 1, BW * 0.5, 0.0).astype(np.float32)
    np.maximum.at(bias, l_idx, l_val)
    np.maximum.at(bias, r_idx, r_val)
    np.maximum.at(bias, beats, np.float32(BW))
    return bias


def _temporal_bias():
    scale = (TA - 1) / (TM - 1)
    audio_pos = np.arange(TM, dtype=np.float32) * scale
    diff = audio_pos[:, None] - np.arange(TA, dtype=np.float32)[None, :]
    return (-(diff ** 2) / (2.0 * SIGMA ** 2)).astype(np.float32)


def _chunk_w(w, n_out_chunks, n_in_chunks, dt=np.float32):
    # w: [E, Dk] row-major -> [ec, p(in), kc, j(out)] (4KB-contiguous rows)
    E, Dk = w.shape
    return np.ascontiguousarray(
        w.reshape(n_out_chunks, E // n_out_chunks, n_in_chunks, Dk // n_in_chunks)
        .transpose(0, 3, 2, 1).astype(dt))


def _pp(vec):
    # [n*128] -> [128, n] per-partition layout
    v = np.asarray(vec, np.float32).reshape(-1, 128)
    return np.ascontiguousarray(v.T)


def kernel(**inputs):
    import ml_dtypes
    from concourse.bass_utils import run_bass_kernel_spmd

    nc = _get_module()
    bf16 = ml_dtypes.bfloat16

    src = np.asarray(inputs["src"], np.float32)
    audio = np.asarray(inputs["audio_memory"], np.float32)
    beats = inputs["beat_frames"]
    f32 = np.float32

    # feature-major: [B, KD, 128, T]
    xin_all = np.ascontiguousarray(
        src.transpose(1, 2, 0).reshape(B, KD, 128, TM))
    ain_all = np.ascontiguousarray(
        audio.transpose(1, 2, 0).reshape(B, KD, 128, TA))

    sa_in_w = np.asarray(inputs["sa_in_w"], f32)
    sa_in_b = np.asarray(inputs["sa_in_b"], f32)
    sa_out_w = np.asarray(inputs["sa_out_w"], f32)
    sa_out_b = np.asarray(inputs["sa_out_b"], f32)
    ca_in_w = np.asarray(inputs["ca_in_w"], f32)
    ca_in_b = np.asarray(inputs["ca_in_b"], f32)
    ca_out_w = np.asarray(inputs["ca_out_w"], f32)
    ca_out_b = np.asarray(inputs["ca_out_b"], f32)
    gate = float(np.asarray(inputs["gate"]))
    tg = float(np.tanh(gate))

    # SA: fold 1/8 score scale into q weights+bias; v-bias into out-proj bias.
    wqk_sa_eff = np.concatenate([sa_in_w[:D] / 8.0, sa_in_w[D:2 * D]], axis=0)
    bqk_sa_eff = np.concatenate([sa_in_b[:D] / 8.0, sa_in_b[D:2 * D]])
    bo_sa_eff = sa_out_b + sa_out_w @ sa_in_b[2 * D:]
    # CA: same folds + tanh(gate) into out-proj weights/bias.
    wqk_ca_eff = np.concatenate([ca_in_w[:D] / 8.0, ca_in_w[D:2 * D]], axis=0)
    bqk_ca_eff = np.concatenate([ca_in_b[:D] / 8.0, ca_in_b[D:2 * D]])
    wo_ca_eff = tg * ca_out_w
    bo_ca_eff = tg * (ca_out_b + ca_out_w @ ca_in_b[2 * D:])

    bias = _temporal_bias() + _beat_bias(beats)[None, :]  # [tq, tk]
    biasT = np.ascontiguousarray(bias.T.reshape(TCH, 128, TM))

    w2_arr = _chunk_w(np.asarray(inputs["lin2_w"], f32), 8, KF, bf16)
    weights = {
        "wqk_sa": _chunk_w(wqk_sa_eff, 16, KD),
        "wv_sa": _chunk_w(sa_in_w[2 * D:], 4, KD),
        "wo_sa": _chunk_w(sa_out_w, 8, KD, bf16),
        "wqk_ca": _chunk_w(wqk_ca_eff, 16, KD),
        "wv_ca": _chunk_w(ca_in_w[2 * D:], 4, KD),
        "wo_ca": _chunk_w(wo_ca_eff, 8, KD, bf16),
        "w1": _chunk_w(np.asarray(inputs["lin1_w"], f32), KF, KD),
        "w2": np.ascontiguousarray(
            w2_arr.reshape(8, 128, 2, KF // 2, 128).transpose(0, 2, 1, 3, 4)),
        "bqk_sa": _pp(bqk_sa_eff),
        "bo_sa": _pp(bo_sa_eff),
        "bqk_ca": _pp(bqk_ca_eff),
        "bo_ca": _pp(bo_ca_eff),
        "b1": _pp(np.asarray(inputs["lin1_b"], f32)),
        "b2": _pp(np.asarray(inputs["lin2_b"], f32)),
        "n1g": _pp(np.asarray(inputs["n1_g"], f32)),
        "n1b": _pp(np.asarray(inputs["n1_b"], f32)),
        "ncg": _pp(np.asarray(inputs["nc_g"], f32)),
        "ncb": _pp(np.asarray(inputs["nc_b"], f32)),
        "n2g": _pp(np.asarray(inputs["n2_g"], f32)),
        "n2b": _pp(np.asarray(inputs["n2_b"], f32)),
        "biasT": biasT,
        "ident": np.eye(128, dtype=f32),
        "onescol": np.ones((128, 1), f32),
        "onesrow": np.ones((1, 128), f32),
    }

    in_maps = []
    for c in range(NCORES):
        m = dict(weights)
        m["xin"] = np.ascontiguousarray(xin_all[BPC * c:BPC * (c + 1)])
        m["ain"] = np.ascontiguousarray(ain_all[BPC * c:BPC * (c + 1)])
        in_maps.append(m)

    res = run_bass_kernel_spmd(nc, in_maps, core_ids=list(range(NCORES)))
    outs = [r["out"] for r in res.results]  # each [BPC, KD, 128, TM]
    full = np.concatenate(outs, axis=0)     # [B, KD, 128, TM]
    return np.ascontiguousarray(
        full.reshape(B, D, TM).transpose(2, 0, 1)).astype(np.float32)
